# revision 29
# baseline (speedup 1.0000x reference)
"""Trainium2 Bass kernel for nn_CrossAxisAttention (stripe attention block).

Reference computation (per batch image, C=256, H=W=56):
  qkv = 1x1conv(x); q,k,v = split(qkv)
  v   = v + dwconv3x3(v)
  heads 0-3: attention within 7-row horizontal stripes
  heads 4-7: attention within 7-col vertical stripes
  y   = 1x1conv(concat_heads)

Sharding: pure data-parallel, one batch image per NeuronCore (B=8 = 8 cores).

Per-core plan, all matmuls in bf16 (1 cycle/row on the PE vs fp32's 4),
fp32 PSUM accumulation:
  - x split across the qSP and qActivation HWDGE queues (first 448-col
    chunk separately so the qkv matmuls start early); constants ride the
    gpsimd SWDGE queue
  - dwconv diagonal-weight tiles built on device (ident * w9 per tap) to
    avoid shipping 600KB of mostly-zero matrices
  - qkv / dwconv / proj: dense bf16 matmuls, PSUM bias-evacs alternating
    between the vector and scalar engines
  - attention per (branch, stripe) unit, k-token chunks of 98 (392 = 4*98):
      v^T chunks transposed into a gapped [98, 4*34] layout whose 33rd
      column per head is a constant ones-column: the AV matmuls (M=34, two
      per PSUM tile at partition bases 0/64) then produce the softmax
      denominators as a free extra partition row, so no separate
      ones-matmul accumulation pass is needed
      logits^T per chunk: 4 row-tiled bf16 matmuls, the two head-pair
      PSUM tiles interleaved so all 4 stream concurrently on the PE
      exp via 2 ACT ops per chunk (head-pairs), bf16 output, scale folded
      in (max-subtraction skipped: logits are O(0.5))
      normalize: ACT evacs of the two AV tiles, select-broadcast matmuls
      mapping each head's sums row to its 32 channels, one
      reciprocal_approx_fast, 4 DVE multiplies
      the per-stripe tail is software-pipelined into the next stripe so the
      PE never waits on the ACT/DVE normalize chain
  - W branch first, H stripes in order [1..7,0], proj t-tiles interleaved
    into the H loop (sharing the bcast PSUM bank) so only proj t0 trails

  Dense qkv/proj matmuls are split into two M=64 column-halves at PSUM
  bases {0,64} so the halves stream concurrently, hiding pipeline fills.

  Perf context: the per-core HAM/power manager grants ~41us of 2.4GHz PE
  then clamps to 1.2GHz (K=4/8) while the PE stays saturated; the kernel is
  PE-bound (~96% busy in the attention phase), so wall ~= 2x full-clock PE
  time minus the grace window. HW exec ~234-239us/core (from 768us fp32
  baseline).
"""

import numpy as np
import ml_dtypes
from contextlib import ExitStack

import concourse.bass as bass
import concourse.bacc as bacc
import concourse.mybir as mybir
import concourse.tile as tile

F32 = mybir.dt.float32
BF16 = mybir.dt.bfloat16
EXPF = mybir.ActivationFunctionType.Exp
IDF = mybir.ActivationFunctionType.Identity

C = 256
HW = 56
T = HW * HW          # 3136
SW = 7
NS = HW // SW        # 8 stripes
STR = SW * HW        # 392 tokens per stripe
KC = 98              # k-token chunk (392 = 4*98)
NCHUNK = 4
SCALE = 32 ** -0.5   # head_dim = 32
NT = 7               # token tiles of 448 for the dense matmuls
TT = T // NT         # 448
XQ = 4               # x DMA quarters
G = 34               # per-head AV group: 32 chans + sums row + pad (even stride)


def build_module():
    nc = bacc.Bacc(None)
    x_d = nc.dram_tensor("x", [C, T], BF16, kind="ExternalInput")
    wqkvT_d = nc.dram_tensor("wqkvT", [C, 3 * C], BF16, kind="ExternalInput")
    bq_d = nc.dram_tensor("bq", [128, 6], F32, kind="ExternalInput")
    w9_d = nc.dram_tensor("w9", [128, 18], F32, kind="ExternalInput")
    ident_d = nc.dram_tensor("ident", [128, 128], BF16, kind="ExternalInput")
    sel_d = nc.dram_tensor("sel", [128, 64], BF16, kind="ExternalInput")
    bdw_d = nc.dram_tensor("bdw", [128, 2], F32, kind="ExternalInput")
    wprojT_d = nc.dram_tensor("wprojT", [C, C], BF16, kind="ExternalInput")
    bp_d = nc.dram_tensor("bp", [128, 2], F32, kind="ExternalInput")
    y_d = nc.dram_tensor("y", [C, T], F32, kind="ExternalOutput")

    with ExitStack() as ctx:
        tc = ctx.enter_context(tile.TileContext(nc))
        _body(ctx, tc, x_d, wqkvT_d, bq_d, w9_d, ident_d, sel_d, bdw_d,
              wprojT_d, bp_d, y_d)
    if not nc.is_finalized():
        nc.finalize()
    return nc


def _body(ctx, tc, x_d, wqkvT_d, bq_d, w9_d, ident_d, sel_d, bdw_d,
          wprojT_d, bp_d, y_d):
    nc = tc.nc

    const_p = ctx.enter_context(tc.tile_pool(name="const", bufs=1))
    big_p = ctx.enter_context(tc.tile_pool(name="big", bufs=4))
    qkv_p = ctx.enter_context(tc.tile_pool(name="qkv", bufs=6))
    vp_p = ctx.enter_context(tc.tile_pool(name="vp", bufs=2))
    e_p = ctx.enter_context(tc.tile_pool(name="epool", bufs=8))
    vt_p = ctx.enter_context(tc.tile_pool(name="vtp", bufs=1))
    sum_p = ctx.enter_context(tc.tile_pool(name="sums", bufs=6))
    rb_p = ctx.enter_context(tc.tile_pool(name="rbp", bufs=3))
    rep_p = ctx.enter_context(tc.tile_pool(name="rep", bufs=1))
    evac_p = ctx.enter_context(tc.tile_pool(name="evac", bufs=3))
    dram_p = ctx.enter_context(tc.tile_pool(name="drp", bufs=3, space="DRAM"))

    # ---- x on both HWDGE queues first; constants via the gpsimd SWDGE ----
    x_sb = [big_p.tile([128, T], BF16, tag="big", name=f"x{i}") for i in range(2)]
    xsplits = [(0, TT), (TT, 2 * TT), (2 * TT, 4 * TT), (4 * TT, T)]
    for i, (c0, c1) in enumerate(xsplits):
        for kc in range(2):
            eng = nc.sync if kc == 0 else nc.scalar
            eng.dma_start(out=x_sb[kc][:, c0:c1],
                          in_=x_d[128 * kc:128 * (kc + 1), c0:c1])

    wq_sb = []
    wp_sb = []
    for kc in range(2):
        wq = const_p.tile([128, 3 * C], BF16, tag=f"wq{kc}", name=f"wq{kc}")
        nc.gpsimd.dma_start(out=wq[:], in_=wqkvT_d[128 * kc:128 * (kc + 1), :])
        wq_sb.append(wq)
        wp = const_p.tile([128, C], BF16, tag=f"wp{kc}", name=f"wp{kc}")
        nc.gpsimd.dma_start(out=wp[:], in_=wprojT_d[128 * kc:128 * (kc + 1), :])
        wp_sb.append(wp)
    bq_sb = const_p.tile([128, 6], F32)
    nc.gpsimd.dma_start(out=bq_sb[:], in_=bq_d[:, :])
    bdw_sb = const_p.tile([128, 2], F32)
    nc.gpsimd.dma_start(out=bdw_sb[:], in_=bdw_d[:, :])
    bp_sb = const_p.tile([128, 2], F32)
    nc.gpsimd.dma_start(out=bp_sb[:], in_=bp_d[:, :])
    ident = const_p.tile([128, 128], BF16)
    nc.gpsimd.dma_start(out=ident[:], in_=ident_d[:, :])
    sel = const_p.tile([128, 64], BF16)
    nc.gpsimd.dma_start(out=sel[:], in_=sel_d[:, :])
    w9_sb = const_p.tile([128, 18], F32)
    nc.gpsimd.dma_start(out=w9_sb[:], in_=w9_d[:, :])

    # dwconv diag tiles built on device: diag(w9[:, i]) = ident * w9[:, i]
    diag_sb = []
    for i in range(18):
        dg = const_p.tile([128, 128], BF16, tag=f"diag{i}", name=f"diag{i}")
        nc.vector.tensor_scalar_mul(dg[:], ident[:], w9_sb[:, i:i + 1])
        diag_sb.append(dg)

    q_sb = [qkv_p.tile([128, T], BF16, tag="qkv", name=f"q{i}") for i in range(2)]
    k_sb = [qkv_p.tile([128, T], BF16, tag="qkv", name=f"k{i}") for i in range(2)]
    vdw_sb = [qkv_p.tile([128, T], BF16, tag="qkv", name=f"vdw{i}") for i in range(2)]

    # padded v for dwconv: [128, 58, 58] with zero border
    vpad_sb = []
    for cc in range(2):
        vp = vp_p.tile([128, 58 * 58], BF16, tag="vp")
        nc.vector.memset(vp[:], 0.0)
        vpad_sb.append(vp)

    def evac_add(idx, out_ap, ps_ap, bias_ap):
        """PSUM -> SBUF bias-add, alternating vector/scalar engines."""
        if idx % 2 == 0:
            nc.vector.tensor_scalar_add(out_ap, ps_ap, bias_ap)
        else:
            nc.scalar.activation(out_ap, ps_ap, IDF, bias=bias_ap)

    # ---- phase A: qkv matmul  [768,256] @ [256,3136] ----
    ei = 0
    with tc.tile_pool(name="ps_a", bufs=8, space="PSUM") as ps_a:
        for t in range(NT):
            for m in range(6):
                ps = ps_a.tile([128, TT], F32, tag="ps", padded_shape=[128, 512])
                # two M=64 column-halves at bases {0,64}: overlapping PE tiles
                for kc in range(2):
                    for hf in range(2):
                        nc.tensor.matmul(
                            ps[64 * hf:64 * (hf + 1), :],
                            wq_sb[kc][:, 128 * m + 64 * hf:128 * m + 64 * (hf + 1)],
                            x_sb[kc][:, TT * t:TT * (t + 1)],
                            start=(kc == 0), stop=(kc == 1),
                        )
                bias = bq_sb[:, m:m + 1]
                if m < 2:
                    evac_add(ei, q_sb[m][:, TT * t:TT * (t + 1)], ps[:], bias)
                elif m < 4:
                    evac_add(ei, k_sb[m - 2][:, TT * t:TT * (t + 1)], ps[:], bias)
                else:
                    cc = m - 4
                    vp3 = vpad_sb[cc][:].rearrange("p (h w) -> p h w", h=58)
                    out_ap = vp3[:, 1 + 8 * t:1 + 8 * (t + 1), 1:57]
                    ps3 = ps[:].rearrange("p (a b) -> p a b", a=8)
                    evac_add(ei, out_ap, ps3, bias)
                ei += 1

        # ---- phase B: depthwise 3x3 as 9 diagonal matmuls (W half first:
        # the W branch of attention runs first and needs vdw[1]) ----
        for cc in (1, 0):
            diags = diag_sb[9 * cc:9 * (cc + 1)]
            vp3 = vpad_sb[cc][:].rearrange("p (h w) -> p h w", h=58)
            for t in range(NT):
                ps = ps_a.tile([128, TT], F32, tag="ps", padded_shape=[128, 512])
                ps3 = ps[:].rearrange("p (a b) -> p a b", a=8)
                for tap in range(9):
                    dh, dw = divmod(tap, 3)
                    rhs = vp3[:, 8 * t + dh:8 * t + dh + 8, dw:dw + 56]
                    nc.tensor.matmul(
                        ps3, diags[tap][:], rhs,
                        start=(tap == 0), stop=(tap == 8),
                    )
                evac_add(ei, vdw_sb[cc][:, TT * t:TT * (t + 1)], ps[:],
                         bdw_sb[:, cc:cc + 1])
                ei += 1

    attn_sb = [big_p.tile([128, T], BF16, tag="big", name=f"attn{i}")
               for i in range(2)]

    # ---- phase C: stripe attention ----
    # W-branch stripes of k/v_dw need contiguous repacks; emit all upfront,
    # gpsimd streams them while the H branch runs on the PE
    k3w = k_sb[1][:].rearrange("p (h w) -> p h w", h=HW)
    v3w = vdw_sb[1][:].rearrange("p (h w) -> p h w", h=HW)
    kw_rep = []
    vw_rep = []
    for s in range(NS):
        kw_s = rep_p.tile([128, STR], BF16, tag=f"kw{s}", name=f"kw{s}")
        nc.gpsimd.tensor_copy(kw_s[:], k3w[:, :, SW * s:SW * (s + 1)])
        kw_rep.append(kw_s)
    for s in range(NS):
        vw_s = rep_p.tile([128, STR], BF16, tag=f"vw{s}", name=f"vw{s}")
        nc.gpsimd.tensor_copy(vw_s[:], v3w[:, :, SW * s:SW * (s + 1)])
        vw_rep.append(vw_s)

    with (
        tc.tile_pool(name="ps_lg", bufs=2, space="PSUM") as ps_lg,
        tc.tile_pool(name="ps_vt", bufs=1, space="PSUM") as ps_vt,
        tc.tile_pool(name="ps_av", bufs=1, space="PSUM") as ps_av,
        tc.tile_pool(name="ps_bc", bufs=1, space="PSUM") as ps_bc,
    ):
        # AV tiles allocated once; unused partitions zeroed so the ACT sums
        # evac stays finite (select-bcast matmul multiplies them by 0)
        av_ps = []
        for i in range(2):
            av = ps_av.tile([128, 512], F32, tag=f"av{i}", name=f"av{i}")
            nc.vector.memset(av[:], 0.0)
            av_ps.append(av)
        # v^T ping-pong tiles: ones pre-set in each head's 33rd column
        vt_sb = []
        for i in range(2):
            vt = vt_p.tile([128, NCHUNK * 4 * G], BF16, tag=f"vt{i}", name=f"vt{i}")
            nc.vector.memset(vt[:], 1.0)
            vt_sb.append(vt)

        pending_tail = [None]

        def emit_proj(t):
            """proj t-tile (both output halves); shares the bc PSUM bank."""
            for m in range(2):
                ps = ps_bc.tile([128, 512], F32, tag="bc")
                for kc in range(2):
                    for hf in range(2):
                        nc.tensor.matmul(
                            ps[64 * hf:64 * (hf + 1), 0:TT],
                            wp_sb[kc][:, 128 * m + 64 * hf:128 * m + 64 * (hf + 1)],
                            attn_sb[kc][:, TT * t:TT * (t + 1)],
                            start=(kc == 0), stop=(kc == 1),
                        )
                st = evac_p.tile([128, TT], F32, tag="st")
                nc.vector.tensor_scalar_add(st[:], ps[:, 0:TT], bp_sb[:, m:m + 1])
                eng = nc.sync if (t + m) % 2 == 0 else nc.scalar
                eng.dma_start(
                    out=y_d[128 * m:128 * (m + 1), TT * t:TT * (t + 1)], in_=st[:])

        # W branch first so the proj tiles (needing every W stripe) can
        # interleave into the H loop; H stripes run [1..7, 0] so the
        # column-range needed by each proj tile completes early and only
        # proj t0 remains after the last (s=0) stripe
        emitted_proj = [False] * NT
        prev_stripe = [None]

        def try_emit_projs(tails_done):
            done = set(tails_done)
            for t in range(NT):
                if emitted_proj[t]:
                    continue
                need = set(range(TT * t // STR,
                                 (TT * (t + 1) - 1) // STR + 1))
                if need <= done:
                    emit_proj(t)
                    emitted_proj[t] = True
        for cc in (1, 0):
            q3 = q_sb[cc][:].rearrange("p (h w) -> p h w", h=HW)
            a3 = attn_sb[cc][:].rearrange("p (h w) -> p h w", h=HW)
            s_order = list(range(NS)) if cc == 1 else [1, 2, 3, 4, 5, 6, 7, 0]
            tails_done = []
            for s in s_order:
                if cc == 0:
                    k_src = k_sb[0][:]
                    v_src = vdw_sb[0][:]
                    base = STR * s
                else:
                    k_src = kw_rep[s][:]
                    v_src = vw_rep[s][:]
                    base = 0

                def kslice(ap_flat, j, p0, p1):
                    """[p0:p1, KC-chunk-j] AP of stripe s (kernel token order)."""
                    return ap_flat[p0:p1, base + KC * j: base + KC * (j + 1)]

                # transpose v chunks into one PSUM bank, gapped 34-stride
                pvt = ps_vt.tile([128, 512], F32, tag="pvt")
                pvt3 = pvt[:].bitcast(BF16)[:, 0:NCHUNK * 4 * G].rearrange(
                    "p (c h g) -> p c h g", c=NCHUNK, h=4)
                for j in range(NCHUNK):
                    nc.tensor.transpose(
                        pvt3[0:KC, j, :, 0:32], kslice(v_src, j, 0, 128), ident[:])
                vt = vt_sb[s % 2]
                vt3 = vt[:].rearrange("p (c h g) -> p c h g", c=NCHUNK, h=4)
                nc.vector.tensor_copy(vt3[0:KC, :, :, 0:32],
                                      pvt3[0:KC, :, :, 0:32])

                # logits^T + exp, chunk by chunk
                es = []
                for j in range(NCHUNK):
                    e = e_p.tile([128, 4 * STR], BF16, tag="e")
                    lgs = [ps_lg.tile([128, 1024], F32, tag="lg",
                                      name=f"lg{_i}") for _i in range(2)]
                    # interleave the two head-pair tiles so all 4 row-tiled
                    # matmuls are adjacent in the PE queue (4-way streams)
                    for hh in range(2):
                        for hp in range(2):
                            h = 2 * hp + hh
                            if cc == 0:
                                rhs = q_sb[cc][32 * h:32 * (h + 1),
                                               STR * s:STR * (s + 1)]
                            else:
                                rhs = q3[32 * h:32 * (h + 1), :, SW * s:SW * (s + 1)]
                            nc.tensor.matmul(
                                lgs[hp][0:KC, 512 * hh:512 * hh + STR],
                                kslice(k_src, j, 32 * h, 32 * (h + 1)),
                                rhs,
                                start=True, stop=True,
                                tile_position=(32 * h, 0),
                            )
                    for hp in range(2):
                        lgv = lgs[hp][:].rearrange("p (a b) -> p a b", b=512)[0:KC, :, 0:STR]
                        ev = e[:].rearrange("p (a b) -> p a b", b=STR)[0:KC, 2 * hp:2 * hp + 2, :]
                        nc.scalar.activation(ev, lgv, EXPF, scale=SCALE)
                    es.append(e)
                    if j == 0 and pending_tail[0] is not None:
                        pending_tail[0]()
                        pending_tail[0] = None
                        if cc == 0 and prev_stripe[0] is not None:
                            tails_done.append(prev_stripe[0])
                            prev_stripe[0] = None
                        if cc == 0:
                            try_emit_projs(tails_done)

                # AV with fused sums row: M=34 groups at bases {0, 64}
                for j in range(NCHUNK):
                    for h in range(4):
                        nc.tensor.matmul(
                            av_ps[h // 2][64 * (h % 2):64 * (h % 2) + G, 0:STR],
                            vt[0:KC, (4 * G) * j + G * h:(4 * G) * j + G * h + G],
                            es[j][0:KC, STR * h:STR * (h + 1)],
                            start=(j == 0), stop=(j == NCHUNK - 1),
                        )

                def tail(cc=cc, s=s, a3=a3):
                    # sums evac, select-bcast matmuls, reciprocal, normalize
                    ssb = []
                    for i in range(2):
                        sb = sum_p.tile([128, STR], BF16, tag="ssb")
                        nc.vector.tensor_copy(sb[:], av_ps[i][:, 0:STR])
                        ssb.append(sb)
                    bc = ps_bc.tile([128, 512], F32, tag="bc")
                    for i in range(2):
                        nc.tensor.matmul(bc[64 * i:64 * (i + 1), 0:STR],
                                         sel[:], ssb[i][:],
                                         start=True, stop=True)
                    rb = rb_p.tile([128, STR], F32, tag="rb")
                    nc.vector.reciprocal_approx_fast(out=rb[:], in_=bc[:, 0:STR])
                    for h in range(4):
                        avh = av_ps[h // 2][64 * (h % 2):64 * (h % 2) + 32, 0:STR]
                        rbh = rb[32 * h:32 * (h + 1), :]
                        if cc == 0:
                            nc.vector.tensor_mul(
                                attn_sb[0][32 * h:32 * (h + 1),
                                           STR * s:STR * (s + 1)], avh, rbh)
                        else:
                            av3 = avh.rearrange("p (a b) -> p a b", a=HW)
                            rb3 = rbh.rearrange("p (a b) -> p a b", a=HW)
                            nc.vector.tensor_mul(
                                a3[32 * h:32 * (h + 1), :, SW * s:SW * (s + 1)],
                                av3, rb3)

                if cc == 0 and s == s_order[-1]:
                    tail()
                    tails_done.append(s)
                else:
                    pending_tail[0] = tail
                    prev_stripe[0] = s if cc == 0 else None

                # proj tiles whose attn columns are fully normalized (tails
                # emitted for every H stripe overlapping the column window)
                if cc == 0:
                    try_emit_projs(tails_done)

        if pending_tail[0] is not None:
            pending_tail[0]()
            pending_tail[0] = None
        for t in range(NT):
            if not emitted_proj[t]:
                emit_proj(t)
                emitted_proj[t] = True


_NC_CACHE = {}


def get_module():
    if "nc" not in _NC_CACHE:
        _NC_CACHE["nc"] = build_module()
    return _NC_CACHE["nc"]


def make_in_maps(x, w_qkv, b_qkv, w_dw, b_dw, w_proj, b_proj):
    B = x.shape[0]
    f = np.float32
    bf = ml_dtypes.bfloat16
    wqkvT = np.ascontiguousarray(np.asarray(w_qkv, dtype=f).T).astype(bf)
    wprojT = np.ascontiguousarray(np.asarray(w_proj, dtype=f).T).astype(bf)
    w9raw = np.ascontiguousarray(np.asarray(w_dw, dtype=f).reshape(C, 9)).copy()
    w9raw[:, 4] += 1.0                                  # fold "+v" residual
    w9 = np.zeros((128, 18), dtype=f)
    for cc in range(2):
        w9[:, 9 * cc:9 * (cc + 1)] = w9raw[128 * cc:128 * (cc + 1), :]
    ident = np.eye(128, dtype=f).astype(bf)
    # sel[k, m] = 1 iff k == 32 + 64*(m//32): maps each AV tile's sums rows
    # (partitions 32 / 96) onto that tile's two 32-channel head groups
    selm = np.zeros((128, 64), dtype=f)
    marr = np.arange(64)
    selm[32 + 64 * (marr // 32), marr] = 1.0
    selm = selm.astype(bf)
    bq = np.ascontiguousarray(np.asarray(b_qkv, dtype=f).reshape(6, 128).T)
    bdw = np.ascontiguousarray(np.asarray(b_dw, dtype=f).reshape(2, 128).T)
    bp = np.ascontiguousarray(np.asarray(b_proj, dtype=f).reshape(2, 128).T)
    x2 = np.ascontiguousarray(np.asarray(x, dtype=f).reshape(B, C, T)).astype(bf)
    return [
        {"x": x2[b], "wqkvT": wqkvT, "bq": bq, "w9": w9, "ident": ident,
         "sel": selm, "bdw": bdw, "wprojT": wprojT, "bp": bp}
        for b in range(B)
    ]


def kernel(x, w_qkv, b_qkv, w_dw, b_dw, w_proj, b_proj):
    from concourse.bass_utils import run_bass_kernel_spmd
    x = np.asarray(x)
    B = x.shape[0]
    in_maps = make_in_maps(np.asarray(x), np.asarray(w_qkv), np.asarray(b_qkv),
                           np.asarray(w_dw), np.asarray(b_dw),
                           np.asarray(w_proj), np.asarray(b_proj))
    nc = get_module()
    br = run_bass_kernel_spmd(nc, in_maps, list(range(B)))
    y = np.stack([br.results[b]["y"] for b in range(B)])
    return y.reshape(B, C, HW, HW).astype(np.float32)
